# revision 2
# baseline (speedup 1.0000x reference)
"""Trainium2 Bass kernel for nn_JacobianGluer (gnn_message_passing).

out[m, o] = sum_n w(|x_m - p_n|^2) * (values[n, o] + (x_m - p_n) . J_n[o, :])

with bump weight w(d2) = exp(1 - 1/(1 - d2/r2)) inside radius, 0 outside.

Strategy (data-parallel over M across 8 cores, everything fused on-chip):
  g[n,m]  = 1 - d2/r2           via one K=5 PE matmul of host-augmented operands
  t       = max(g, 1/64)        DVE tensor_scalar (PSUM -> SBUF)
  r       = ~1/t                DVE reciprocal_approx_fast (single custom op)
  w       = exp(1 - r)          ACT Exp with scale=-1, bias=1
  B[c,m] += G[n,c]^T w[n,m]     PE matmul accumulated over n-tiles,
                                G[n, 4k+o] = [J_n[o,k] (k<3) | values - p.J]
  out[m,o] = sum_k xaug[m,k] B[4k+o, m]   via PE transpose + DVE combine
"""

import sys
from contextlib import ExitStack

import numpy as np

sys.path.insert(0, "/opt/trn_rl_repo")

import concourse.bass as bass  # noqa: E402
import concourse.mybir as mybir  # noqa: E402
import concourse.tile as tile  # noqa: E402
from concourse import bacc, bass_utils  # noqa: E402
from concourse.masks import make_identity  # noqa: E402

N_CORES = 8
M, N, D, O = 16384, 4096, 3, 4
MS = M // N_CORES          # 2048 m rows per core
NT = N // 128              # 32 n-tiles
NMT = MS // 128            # 16 m-tiles per core
C = 16                     # channels = 4 k-slots x 4 outputs
R2 = 6.25
EPS = 1.0 / 64.0
F32 = mybir.dt.float32

_CACHE = {}


def _build_bass():
    nc = bacc.Bacc(
        "TRN2",
        target_bir_lowering=False,
        debug=False,
        enable_asserts=False,
        num_devices=N_CORES,
    )
    A = nc.dram_tensor("A", [5, N], F32, kind="ExternalInput").ap()
    B = nc.dram_tensor("B", [5, MS], F32, kind="ExternalInput").ap()
    G = nc.dram_tensor("G", [128, NT * C], F32, kind="ExternalInput").ap()
    XR = nc.dram_tensor("XR", [128, NMT * C], F32, kind="ExternalInput").ap()
    OUT = nc.dram_tensor("OUT", [MS, O], F32, kind="ExternalOutput").ap()

    with tile.TileContext(nc) as tc, ExitStack() as ctx:
        const_pool = ctx.enter_context(tc.tile_pool(name="const", bufs=1))
        pg_pool = ctx.enter_context(tc.tile_pool(name="pg", bufs=2, space="PSUM"))
        po_pool = ctx.enter_context(tc.tile_pool(name="po", bufs=4, space="PSUM"))
        t_pool = ctx.enter_context(tc.tile_pool(name="t", bufs=3))
        r_pool = ctx.enter_context(tc.tile_pool(name="r", bufs=3))
        w_pool = ctx.enter_context(tc.tile_pool(name="w", bufs=3))

        A_sb = const_pool.tile([5, N], F32)
        nc.sync.dma_start(A_sb[:], A[:, :])
        B_sb = const_pool.tile([5, MS], F32)
        nc.sync.dma_start(B_sb[:], B[:, :])
        G_sb = const_pool.tile([128, NT * C], F32)
        nc.sync.dma_start(G_sb[:], G[:, :])
        XR_sb = const_pool.tile([128, NMT * C], F32)
        nc.sync.dma_start(XR_sb[:], XR[:, :])
        ident = const_pool.tile([16, 16], F32)
        make_identity(nc, ident[:])

        # phase C accumulators, one [16, 512] per 512-wide m chunk
        psum_out = [po_pool.tile([16, 512], F32, tag="po", name=f"po{i}") for i in range(MS // 512)]

        MCH = 1024  # m-chunk processed per elementwise instruction
        for nt in range(NT):
            a_sl = A_sb[:, nt * 128:(nt + 1) * 128]       # [5, 128] lhsT
            g_lhsT = G_sb[:, nt * C:(nt + 1) * C]         # [128, 16] lhsT
            for mc in range(MS // MCH):
                pg = pg_pool.tile([128, MCH], F32, tag="pg")
                for h in range(MCH // 512):
                    nc.tensor.matmul(
                        pg[:, h * 512:(h + 1) * 512],
                        a_sl,
                        B_sb[:, mc * MCH + h * 512: mc * MCH + (h + 1) * 512],
                        start=True,
                        stop=True,
                    )
                t = t_pool.tile([128, MCH], F32, tag="t")
                nc.vector.tensor_scalar_max(t[:], pg[:], EPS)
                r = r_pool.tile([128, MCH], F32, tag="r")
                nc.vector.reciprocal_approx_fast(out=r[:], in_=t[:])
                w = w_pool.tile([128, MCH], F32, tag="w")
                nc.scalar.activation(
                    w[:], r[:], mybir.ActivationFunctionType.Exp,
                    bias=1.0, scale=-1.0,
                )
                for h in range(MCH // 512):
                    m4 = (mc * MCH + h * 512) // 512
                    nc.tensor.matmul(
                        psum_out[m4][:, :],
                        g_lhsT,
                        w[:, h * 512:(h + 1) * 512],
                        start=(nt == 0),
                        stop=(nt == NT - 1),
                    )

        # epilogue: B[c, m] -> out[m, o] = sum_k xaug[m, k] * B[4k+o, m]
        stage = const_pool.tile([16, MS], F32)
        for m4 in range(MS // 512):
            nc.vector.tensor_copy(stage[:, m4 * 512:(m4 + 1) * 512], psum_out[m4][:])
        prod = const_pool.tile([128, NMT * C], F32)
        for mt in range(NMT):
            pt = po_pool.tile([128, 16], F32, tag="po", name=f"pt{mt}")
            nc.tensor.matmul(
                pt[:], stage[:, mt * 128:(mt + 1) * 128], ident[:],
                start=True, stop=True,
            )
            nc.vector.tensor_mul(
                prod[:, mt * C:(mt + 1) * C], pt[:], XR_sb[:, mt * C:(mt + 1) * C]
            )
        # sum the 4 k-groups: channels are k-major (c = 4k + o)
        s1 = const_pool.tile([128, NMT * 8], F32)
        p3 = prod[:].rearrange("p (t c) -> p t c", c=16)
        s13 = s1[:].rearrange("p (t c) -> p t c", c=8)
        nc.vector.tensor_add(s13, p3[:, :, 0:8], p3[:, :, 8:16])
        s2 = const_pool.tile([128, NMT * 4], F32)
        s23 = s2[:].rearrange("p (t c) -> p t c", c=4)
        nc.vector.tensor_add(s23, s13[:, :, 0:4], s13[:, :, 4:8])
        nc.sync.dma_start(
            OUT.rearrange("(t p) o -> p t o", p=128),
            s2[:].rearrange("p (t o) -> p t o", o=4),
        )

    nc.compile()
    return nc


def _host_prep(x, points, jacobians, values):
    x = np.ascontiguousarray(np.asarray(x, np.float32))
    points = np.ascontiguousarray(np.asarray(points, np.float32))
    jacobians = np.ascontiguousarray(np.asarray(jacobians, np.float32))
    values = np.ascontiguousarray(np.asarray(values, np.float32))

    A = np.empty((5, N), np.float32)
    A[0:3] = (2.0 / R2) * points.T
    A[3] = 1.0
    A[4] = -(points * points).sum(1) / R2

    B = np.empty((5, M), np.float32)
    B[0:3] = x.T
    B[3] = 1.0 - (x * x).sum(1) / R2
    B[4] = 1.0

    pJ = np.einsum("nd,nod->no", points, jacobians)
    G = np.empty((N, C), np.float32)
    for k in range(3):
        G[:, 4 * k:4 * k + 4] = jacobians[:, :, k]
    G[:, 12:16] = values - pJ
    G_sb = np.ascontiguousarray(
        G.reshape(NT, 128, C).transpose(1, 0, 2).reshape(128, NT * C)
    )

    in_maps = []
    for c in range(N_CORES):
        xs = x[c * MS:(c + 1) * MS]
        xa = np.concatenate([xs, np.ones((MS, 1), np.float32)], 1)
        xr = np.ascontiguousarray(
            np.broadcast_to(xa.reshape(NMT, 128, 4, 1), (NMT, 128, 4, 4))
            .transpose(1, 0, 2, 3)
            .reshape(128, NMT * C)
            .astype(np.float32)
        )
        in_maps.append(
            {
                "A": A,
                "B": np.ascontiguousarray(B[:, c * MS:(c + 1) * MS]),
                "G": G_sb,
                "XR": xr,
            }
        )
    return in_maps


def _run(inputs, trace=False):
    if "nc" not in _CACHE:
        _CACHE["nc"] = _build_bass()
    nc = _CACHE["nc"]
    in_maps = _host_prep(**inputs)
    res = bass_utils.run_bass_kernel_spmd(
        nc, in_maps, list(range(N_CORES)), trace=trace
    )
    out = np.concatenate([res.results[c]["OUT"] for c in range(N_CORES)], axis=0)
    return out, res


def kernel(x, points, jacobians, values):
    out, _ = _run(dict(x=x, points=points, jacobians=jacobians, values=values))
    return out


# revision 11
# speedup vs baseline: 1.1612x; 1.1612x over previous
"""Trainium2 Bass kernel for nn_JacobianGluer (gnn_message_passing).

out[m, o] = sum_n w(|x_m - p_n|^2) * (values[n, o] + (x_m - p_n) . J_n[o, :])

with bump weight w(d2) = exp(1 - 1/(1 - d2/r2)) inside radius, 0 outside.

Strategy (data-parallel over M across 8 cores, everything fused on-chip):
  g[n,m]  = 1 - d2/r2           via one K=20 PE matmul (bf16 hi/lo split of the
                                host-augmented K=5 operands; fp32 streams 4x slower)
  t       = max(g, 1/64)        DVE tensor_scalar (PSUM -> SBUF)
  r       = ~1/t                DVE reciprocal_approx_fast (single custom op)
  w       = exp(1 - r)          ACT Exp with scale=-1, bias=1
  B[m,c] += w[n,m]^T G[n,c]     PE matmul, w stationary (LDW is col-count bound,
                                dtype-free) streaming the 16-col G; output lands
                                [m-part, chan] so no transpose epilogue needed.
                                G[n, 4k+o] = [J_n[o,k] (k<3) | values - p.J]
  out[m,o] = sum_k xaug[m,k] B[m, 4k+o]   via one DVE mul + 2 strided adds
"""

import sys
from contextlib import ExitStack

import numpy as np

sys.path.insert(0, "/opt/trn_rl_repo")

import concourse.bass as bass  # noqa: E402
import concourse.mybir as mybir  # noqa: E402
import concourse.tile as tile  # noqa: E402
import ml_dtypes  # noqa: E402
from concourse import bacc, bass_utils  # noqa: E402

N_CORES = 8
M, N, D, O = 16384, 4096, 3, 4
MS = M // N_CORES          # 2048 m rows per core
NT = N // 128              # 32 n-tiles
NMT = MS // 128            # 16 m-tiles per core
C = 16                     # channels = 4 k-slots x 4 outputs
R2 = 6.25
EPS = 1.0 / 64.0
F32 = mybir.dt.float32
BF16 = mybir.dt.bfloat16
KA = 20                    # phase-A contraction: hi/lo split of 5 aug rows x 4 product terms

_CACHE = {}


def _build_bass():
    nc = bacc.Bacc(
        "TRN2",
        target_bir_lowering=False,
        debug=False,
        enable_asserts=False,
        num_devices=N_CORES,
    )
    A = nc.dram_tensor("A", [KA, N], BF16, kind="ExternalInput").ap()
    B = nc.dram_tensor("B", [KA, MS], BF16, kind="ExternalInput").ap()
    G = nc.dram_tensor("G", [128, NT * C], F32, kind="ExternalInput").ap()
    XR = nc.dram_tensor("XR", [128, NMT * C], F32, kind="ExternalInput").ap()
    OUT = nc.dram_tensor("OUT", [MS, O], F32, kind="ExternalOutput").ap()

    with tile.TileContext(nc) as tc, ExitStack() as ctx:
        const_pool = ctx.enter_context(tc.tile_pool(name="const", bufs=1))
        pg_pool = ctx.enter_context(tc.tile_pool(name="pg", bufs=3, space="PSUM"))
        acc_pool = ctx.enter_context(tc.tile_pool(name="acc", bufs=1, space="PSUM"))
        t_pool = ctx.enter_context(tc.tile_pool(name="t", bufs=3))
        r_pool = ctx.enter_context(tc.tile_pool(name="r", bufs=3))
        w_pool = ctx.enter_context(tc.tile_pool(name="w", bufs=3))

        A_sb = const_pool.tile([KA, N], BF16)
        nc.sync.dma_start(A_sb[:], A[:, :])
        B_sb = const_pool.tile([KA, MS], BF16)
        nc.sync.dma_start(B_sb[:], B[:, :])
        G_sb = const_pool.tile([128, NT * C], F32)
        nc.sync.dma_start(G_sb[:], G[:, :])
        XR_sb = const_pool.tile([128, NMT * C], F32)
        nc.sync.dma_start(XR_sb[:], XR[:, :])

        # phase C accumulator: B[m, c] for all 16 m-tiles, [128, 256] in one bank
        psum_acc = acc_pool.tile([128, NMT * C], F32)

        MCH = 1024  # m-chunk processed per elementwise instruction
        for nt in range(NT):
            a_sl = A_sb[:, nt * 128:(nt + 1) * 128]       # [KA, 128] lhsT
            g_rhs = G_sb[:, nt * C:(nt + 1) * C]          # [128, 16] streamed
            for mc in range(MS // MCH):
                pg = pg_pool.tile([128, MCH], F32, tag="pg")
                for h in range(MCH // 512):
                    nc.tensor.matmul(
                        pg[:, h * 512:(h + 1) * 512],
                        a_sl,
                        B_sb[:, mc * MCH + h * 512: mc * MCH + (h + 1) * 512],
                        start=True,
                        stop=True,
                    )
                t = t_pool.tile([128, MCH], F32, tag="t")
                nc.vector.tensor_scalar_max(t[:], pg[:], EPS)
                r = r_pool.tile([128, MCH], F32, tag="r")
                nc.vector.reciprocal_approx_fast(out=r[:], in_=t[:])
                w = w_pool.tile([128, MCH], F32, tag="w")
                nc.scalar.activation(
                    w[:], r[:], mybir.ActivationFunctionType.Exp,
                    bias=1.0, scale=-1.0,
                )
                # phase C, w stationary: B[m, c] += sum_n w[n, m] G[n, c]
                # NOTE: start=True clears the ENTIRE psum bank, and all 16 mt
                # slices share one bank -- so only the very first matmul into
                # the bank may carry start=True; the rest rely on per-element
                # has_written store-vs-add semantics.
                for h in range(MCH // 128):
                    mt = (mc * MCH + h * 128) // 128
                    nc.tensor.matmul(
                        psum_acc[:, mt * C:(mt + 1) * C],
                        w[:, h * 128:(h + 1) * 128],
                        g_rhs,
                        start=(nt == 0 and mt == 0),
                        stop=(nt == NT - 1),
                        skip_group_check=True,
                    )

        # epilogue: out[m, o] = sum_k xaug[m, k] * B[m, 4k+o] (channels k-major)
        prod = const_pool.tile([128, NMT * C], F32)
        nc.vector.tensor_mul(prod[:], psum_acc[:], XR_sb[:])
        s1 = const_pool.tile([128, NMT * 8], F32)
        p3 = prod[:].rearrange("p (t c) -> p t c", c=16)
        s13 = s1[:].rearrange("p (t c) -> p t c", c=8)
        nc.vector.tensor_add(s13, p3[:, :, 0:8], p3[:, :, 8:16])
        s2 = const_pool.tile([128, NMT * 4], F32)
        s23 = s2[:].rearrange("p (t c) -> p t c", c=4)
        nc.vector.tensor_add(s23, s13[:, :, 0:4], s13[:, :, 4:8])
        nc.sync.dma_start(
            OUT.rearrange("(t p) o -> p t o", p=128),
            s2[:].rearrange("p (t o) -> p t o", o=4),
        )

    nc.compile()
    return nc


def _host_prep(x, points, jacobians, values):
    x = np.ascontiguousarray(np.asarray(x, np.float32))
    points = np.ascontiguousarray(np.asarray(points, np.float32))
    jacobians = np.ascontiguousarray(np.asarray(jacobians, np.float32))
    values = np.ascontiguousarray(np.asarray(values, np.float32))

    A = np.empty((5, N), np.float32)
    A[0:3] = (2.0 / R2) * points.T
    A[3] = 1.0
    A[4] = -(points * points).sum(1) / R2

    B = np.empty((5, M), np.float32)
    B[0:3] = x.T
    B[3] = 1.0 - (x * x).sum(1) / R2
    B[4] = 1.0

    # bf16 hi/lo split: g = (ah+al).(bh+bl) via K=20 stacked bf16 contraction
    def _split(v):
        hi = v.astype(ml_dtypes.bfloat16)
        lo = (v - hi.astype(np.float32)).astype(ml_dtypes.bfloat16)
        return hi, lo

    ah, al = _split(A)
    bh, bl = _split(B)
    A2 = np.ascontiguousarray(np.concatenate([ah, al, ah, al], 0))  # [20, N]
    B2 = np.ascontiguousarray(np.concatenate([bh, bh, bl, bl], 0))  # [20, M]

    pJ = np.einsum("nd,nod->no", points, jacobians)
    G = np.empty((N, C), np.float32)
    for k in range(3):
        G[:, 4 * k:4 * k + 4] = jacobians[:, :, k]
    G[:, 12:16] = values - pJ
    G_sb = np.ascontiguousarray(
        G.reshape(NT, 128, C).transpose(1, 0, 2).reshape(128, NT * C)
    )

    in_maps = []
    for c in range(N_CORES):
        xs = x[c * MS:(c + 1) * MS]
        xa = np.concatenate([xs, np.ones((MS, 1), np.float32)], 1)
        xr = np.ascontiguousarray(
            np.broadcast_to(xa.reshape(NMT, 128, 4, 1), (NMT, 128, 4, 4))
            .transpose(1, 0, 2, 3)
            .reshape(128, NMT * C)
            .astype(np.float32)
        )
        in_maps.append(
            {
                "A": A2,
                "B": np.ascontiguousarray(B2[:, c * MS:(c + 1) * MS]),
                "G": G_sb,
                "XR": xr,
            }
        )
    return in_maps


def _run(inputs, trace=False):
    if "nc" not in _CACHE:
        _CACHE["nc"] = _build_bass()
    nc = _CACHE["nc"]
    in_maps = _host_prep(**inputs)
    res = bass_utils.run_bass_kernel_spmd(
        nc, in_maps, list(range(N_CORES)), trace=trace
    )
    out = np.concatenate([res.results[c]["OUT"] for c in range(N_CORES)], axis=0)
    return out, res


def kernel(x, points, jacobians, values):
    out, _ = _run(dict(x=x, points=points, jacobians=jacobians, values=values))
    return out


# revision 15
# speedup vs baseline: 1.9559x; 1.6844x over previous
"""Trainium2 Bass kernel for nn_JacobianGluer (gnn_message_passing).

out[m, o] = sum_n w(|x_m - p_n|^2) * (values[n, o] + (x_m - p_n) . J_n[o, :])

with bump weight w(d2) = exp(1 - 1/(1 - d2/r2)) inside radius, 0 outside.

Strategy (data-parallel over M across 8 cores, everything fused on-chip):
  g[n,m]  = 1 - d2/r2           via one K=20 PE matmul (bf16 hi/lo split of the
                                host-augmented K=5 operands; fp32 streams 4x slower)
  t       = max(g, 1/64)        DVE tensor_scalar (PSUM -> SBUF)
  r       = ~1/t                DVE reciprocal_approx_fast (single custom op)
  w       = exp(1 - r)          ACT Exp with scale=-1, bias=1
  B[m,c] += w[n,m]^T G[n,c]     PE matmul, w stationary (LDW is col-count bound,
                                dtype-free) streaming the 16-col G; output lands
                                [m-part, chan] so no transpose epilogue needed.
                                G[n, 4k+o] = [J_n[o,k] (k<3) | values - p.J]
  out[m,o] = sum_k xaug[m,k] B[m, 4k+o]   via one DVE mul + 2 strided adds
"""

import sys
from contextlib import ExitStack

import numpy as np

sys.path.insert(0, "/opt/trn_rl_repo")

import concourse.bass as bass  # noqa: E402
import concourse.mybir as mybir  # noqa: E402
import concourse.tile as tile  # noqa: E402
import ml_dtypes  # noqa: E402
from concourse import bacc, bass_utils  # noqa: E402
from concourse.masks import make_identity  # noqa: E402

N_CORES = 8
M, N, D, O = 16384, 4096, 3, 4
MS = M // N_CORES          # 2048 m rows per core
NT = N // 128              # 32 n-tiles
NMT = MS // 128            # 16 m-tiles per core
C = 16                     # channels = 4 k-slots x 4 outputs
R2 = 6.25
EPS = 1.0 / 64.0
F32 = mybir.dt.float32
BF16 = mybir.dt.bfloat16
KA = 20                    # phase-A contraction: hi/lo split of 5 aug rows x 4 product terms

_CACHE = {}


def _build_bass():
    nc = bacc.Bacc(
        "TRN2",
        target_bir_lowering=False,
        debug=False,
        enable_asserts=False,
        num_devices=N_CORES,
    )
    A = nc.dram_tensor("A", [KA, N], BF16, kind="ExternalInput").ap()
    B = nc.dram_tensor("B", [KA, MS], BF16, kind="ExternalInput").ap()
    G = nc.dram_tensor("G", [128, NT * C], F32, kind="ExternalInput").ap()
    XR = nc.dram_tensor("XR", [128, NMT * C], F32, kind="ExternalInput").ap()
    OUT = nc.dram_tensor("OUT", [MS, O], F32, kind="ExternalOutput").ap()

    with tile.TileContext(nc) as tc, ExitStack() as ctx:
        const_pool = ctx.enter_context(tc.tile_pool(name="const", bufs=1))
        pg_pool = ctx.enter_context(tc.tile_pool(name="pg", bufs=2, space="PSUM"))
        po_pool = ctx.enter_context(tc.tile_pool(name="po", bufs=4, space="PSUM"))
        t_pool = ctx.enter_context(tc.tile_pool(name="t", bufs=3))
        r_pool = ctx.enter_context(tc.tile_pool(name="r", bufs=3))
        w_pool = ctx.enter_context(tc.tile_pool(name="w", bufs=3))

        A_sb = const_pool.tile([KA, N], BF16)
        nc.sync.dma_start(A_sb[:], A[:, :])
        B_sb = const_pool.tile([KA, MS], BF16)
        nc.sync.dma_start(B_sb[:], B[:, :])
        G_sb = const_pool.tile([128, NT * C], F32)
        nc.sync.dma_start(G_sb[:], G[:, :])
        XR_sb = const_pool.tile([128, NMT * C], F32)
        nc.sync.dma_start(XR_sb[:], XR[:, :])
        ident = const_pool.tile([16, 16], F32)
        make_identity(nc, ident[:])

        # phase C accumulators: one [16 chan, 512 m] bank per 512-wide m chunk
        psum_out = [
            po_pool.tile([16, 512], F32, tag="po", name=f"po{i}")
            for i in range(MS // 512)
        ]

        MCH = 1024  # m-chunk processed per elementwise instruction
        for nt in range(NT):
            a_sl = A_sb[:, nt * 128:(nt + 1) * 128]       # [KA, 128] lhsT
            g_rhs = G_sb[:, nt * C:(nt + 1) * C]          # [128, 16] streamed
            for mc in range(MS // MCH):
                pg = pg_pool.tile([128, MCH], F32, tag="pg")
                for h in range(MCH // 512):
                    nc.tensor.matmul(
                        pg[:, h * 512:(h + 1) * 512],
                        a_sl,
                        B_sb[:, mc * MCH + h * 512: mc * MCH + (h + 1) * 512],
                        start=True,
                        stop=True,
                    )
                t = t_pool.tile([128, MCH], F32, tag="t")
                nc.vector.tensor_scalar_max(t[:], pg[:], EPS)
                r = r_pool.tile([128, MCH], F32, tag="r")
                nc.vector.reciprocal_approx_fast(out=r[:], in_=t[:])
                w = w_pool.tile([128, MCH], F32, tag="w")
                nc.scalar.activation(
                    w[:], r[:], mybir.ActivationFunctionType.Exp,
                    bias=1.0, scale=-1.0,
                )
                # phase C, G stationary (fp32): B[c, m-chunk] += G^T w
                for h in range(MCH // 512):
                    m4 = (mc * MCH + h * 512) // 512
                    nc.tensor.matmul(
                        psum_out[m4][:, :],
                        g_rhs,
                        w[:, h * 512:(h + 1) * 512],
                        start=(nt == 0),
                        stop=(nt == NT - 1),
                    )

        # epilogue: B[c, m] -> out[m, o] = sum_k xaug[m, k] * B[m-layout, 4k+o]
        stage = const_pool.tile([16, MS], F32)
        for m4 in range(MS // 512):
            nc.scalar.copy(stage[:, m4 * 512:(m4 + 1) * 512], psum_out[m4][:])
        prod = const_pool.tile([128, NMT * C], F32)
        for mt in range(NMT):
            pt = po_pool.tile([128, 16], F32, tag="po", name=f"pt{mt}")
            nc.tensor.transpose(pt[:], stage[:, mt * 128:(mt + 1) * 128], ident[:])
            nc.vector.tensor_mul(
                prod[:, mt * C:(mt + 1) * C], pt[:], XR_sb[:, mt * C:(mt + 1) * C]
            )
        s1 = const_pool.tile([128, NMT * 8], F32)
        p3 = prod[:].rearrange("p (t c) -> p t c", c=16)
        s13 = s1[:].rearrange("p (t c) -> p t c", c=8)
        nc.vector.tensor_add(s13, p3[:, :, 0:8], p3[:, :, 8:16])
        s2 = const_pool.tile([128, NMT * 4], F32)
        s23 = s2[:].rearrange("p (t c) -> p t c", c=4)
        nc.vector.tensor_add(s23, s13[:, :, 0:4], s13[:, :, 4:8])
        nc.sync.dma_start(
            OUT.rearrange("(t p) o -> p t o", p=128),
            s2[:].rearrange("p (t o) -> p t o", o=4),
        )

    nc.compile()
    return nc


def _host_prep(x, points, jacobians, values):
    x = np.ascontiguousarray(np.asarray(x, np.float32))
    points = np.ascontiguousarray(np.asarray(points, np.float32))
    jacobians = np.ascontiguousarray(np.asarray(jacobians, np.float32))
    values = np.ascontiguousarray(np.asarray(values, np.float32))

    A = np.empty((5, N), np.float32)
    A[0:3] = (2.0 / R2) * points.T
    A[3] = 1.0
    A[4] = -(points * points).sum(1) / R2

    B = np.empty((5, M), np.float32)
    B[0:3] = x.T
    B[3] = 1.0 - (x * x).sum(1) / R2
    B[4] = 1.0

    # bf16 hi/lo split: g = (ah+al).(bh+bl) via K=20 stacked bf16 contraction
    def _split(v):
        hi = v.astype(ml_dtypes.bfloat16)
        lo = (v - hi.astype(np.float32)).astype(ml_dtypes.bfloat16)
        return hi, lo

    ah, al = _split(A)
    bh, bl = _split(B)
    A2 = np.ascontiguousarray(np.concatenate([ah, al, ah, al], 0))  # [20, N]
    B2 = np.ascontiguousarray(np.concatenate([bh, bh, bl, bl], 0))  # [20, M]

    pJ = np.einsum("nd,nod->no", points, jacobians)
    G = np.empty((N, C), np.float32)
    for k in range(3):
        G[:, 4 * k:4 * k + 4] = jacobians[:, :, k]
    G[:, 12:16] = values - pJ
    G_sb = np.ascontiguousarray(
        G.reshape(NT, 128, C).transpose(1, 0, 2).reshape(128, NT * C)
    )

    in_maps = []
    for c in range(N_CORES):
        xs = x[c * MS:(c + 1) * MS]
        xa = np.concatenate([xs, np.ones((MS, 1), np.float32)], 1)
        xr = np.ascontiguousarray(
            np.broadcast_to(xa.reshape(NMT, 128, 4, 1), (NMT, 128, 4, 4))
            .transpose(1, 0, 2, 3)
            .reshape(128, NMT * C)
            .astype(np.float32)
        )
        in_maps.append(
            {
                "A": A2,
                "B": np.ascontiguousarray(B2[:, c * MS:(c + 1) * MS]),
                "G": G_sb,
                "XR": xr,
            }
        )
    return in_maps


def _run(inputs, trace=False):
    if "nc" not in _CACHE:
        _CACHE["nc"] = _build_bass()
    nc = _CACHE["nc"]
    in_maps = _host_prep(**inputs)
    res = bass_utils.run_bass_kernel_spmd(
        nc, in_maps, list(range(N_CORES)), trace=trace
    )
    out = np.concatenate([res.results[c]["OUT"] for c in range(N_CORES)], axis=0)
    return out, res


def kernel(x, points, jacobians, values):
    out, _ = _run(dict(x=x, points=points, jacobians=jacobians, values=values))
    return out
